# revision 23
# baseline (speedup 1.0000x reference)
"""Luong seq2seq (2-layer BiGRU encoder + attention GRU decoder + vocab
projection) as a single 8-core SPMD Bass/Tile kernel for Trainium2.

Sharding: data-parallel over batch (64 examples -> 8 per core). Each core
runs the full recurrence for its 8 examples and projects onto the full
32000-word vocabulary; the host concatenates per-core logits.

v2 rewrite vs baseline:
- sigmoid(x) = (tanh(x/2)+1)/2 everywhere -> only Tanh/Exp used -> one ACT
  table set, no per-step ACT_TABLE_LOAD thrash.
- gate preact regions packed into ONE PSUM bank at partition offsets
  0/32/64/96 via tile_position col-groups (concurrent PE streams), with
  bufs=2 for cross-step overlap.
- bf16 state + gate intermediates (2x/4x DVE modes), reformulated update
  h = n*(1-z) + z*h with omz/zh computed off the critical chain.
- attention mask added via an identity matmul into the score PSUM group.
- layer outputs accumulated directly into transposed SBUF tiles (l0S,
  hencP) from the per-step transposes: no DRAM roundtrip, no transpose
  phases.
- vocab projection: each out_w chunk DMA'd once (was 3x); 20 chunks kept
  resident in SBUF and consumed as htall m-tiles become ready; dense tail.
- logits stored bf16 (host upcasts).

Self-contained: hardcodes all shapes; takes the full unsharded inputs of
reference.setup_inputs() and returns the full (48, 64, 32000) logits.
"""

import os
import sys
import types

for _p in ("/opt/trn_rl_repo", "/opt/pypackages", "/root/.axon_site",
           "/root/.axon_site/_ro/trn_rl_repo", "/root/.axon_site/_ro/pypackages"):
    if os.path.isdir(_p) and _p not in sys.path:
        sys.path.append(_p)

import numpy as np

from concourse import bass, mybir, tile, bacc
from concourse import bass_utils
from concourse.bass_utils import run_bass_kernel_spmd
from concourse.masks import make_identity

# ---------------------------------------------------------------- constants
V, H, T, B, NCORES = 32000, 512, 48, 64, 8
Bc = B // NCORES            # 8 examples per core
H2, H3 = 2 * H, 3 * H
NSEQ = T * Bc               # 384 (t-major row order: r = t*Bc + b)
NSCAN = T * 2 * Bc          # 768 (enc scan rows: r = t*16 + lane*8 + b)
P = 128
NEG = -1.0e9

f32 = mybir.dt.float32
f32r = mybir.dt.float32r
bf16 = mybir.dt.bfloat16
i32 = mybir.dt.int32
AF = mybir.ActivationFunctionType
OP = mybir.AluOpType

VCHUNKS = [(i * 512, 512) for i in range(62)] + [(62 * 512, 256)]  # 32000
NRES = 20                   # owt chunks kept SBUF-resident through decode


def _install_profile_hook():
    """Make trace=True work: the image's antenv lacks axon_hooks."""
    if "antenv.axon_hooks" in sys.modules:
        return
    try:
        import trn_agent_boot.trn_boot as tb
        hook = tb._ntff_profile_via_ctypes("/opt/axon/libaxon_pjrt.so")
        m = types.ModuleType("antenv.axon_hooks")
        m.get_axon_ntff_profile_hook = lambda: hook
        m.set_axon_ntff_profile_hook = lambda h: None
        sys.modules["antenv.axon_hooks"] = m
        import antenv
        antenv.axon_hooks = m
        bass_utils.upload_artifacts = lambda d: d
    except Exception:
        pass


# ---------------------------------------------------------------- program
def build_program(dbg=False):
    nc = bacc.Bacc("TRN2", target_bir_lowering=False, debug=False,
                   num_devices=NCORES)

    def din(name, shape, dt=f32r):
        return nc.dram_tensor(name, list(shape), dt, kind="ExternalInput").ap()

    io = {}
    io["xeT_in"] = din("xeT_in", (H, NSCAN))
    io["xdT_in"] = din("xdT_in", (H, NSEQ))
    io["amask"] = din("amask", (Bc, NSEQ))
    for name, shape in [
        ("u0", (H, H3)), ("w1t", (H2, H3)),
        ("u1", (H, H3)), ("fct", (H2, H)), ("was", (H2, H)),
        ("wcc", (H2, H)), ("wch", (H, H)),
        ("whd", (H, H3)), ("ud", (H, H3)),
    ]:
        io[name] = din(name, shape, bf16)
    for name, shape in [
        ("w0t", (H, H3)), ("wxd", (H, H3)),
        ("b0", (1, H3)), ("bn0", (1, H)), ("b1", (1, H3)), ("bn1", (1, H)),
        ("fcb", (1, H)), ("bd", (1, H3)), ("bnd", (1, H)),
    ]:
        io[name] = din(name, shape)
    io["owt"] = din("owt", (H, V), bf16)
    io["out"] = nc.dram_tensor("out", [NSEQ, V], bf16,
                               kind="ExternalOutput").ap()
    io["dbg"] = dbg
    if dbg:
        io["dbg_hencP"] = nc.dram_tensor("dbg_hencP", [P, 8 * NSEQ], bf16,
                                         kind="ExternalOutput").ap()
        io["dbg_h0"] = nc.dram_tensor("dbg_h0", [Bc, H], f32,
                                      kind="ExternalOutput").ap()
        io["dbg_htall"] = nc.dram_tensor("dbg_htall", [P, 4 * NSEQ], bf16,
                                         kind="ExternalOutput").ap()
        io["dbg_hall"] = nc.dram_tensor("dbg_hall", [NSEQ, H], f32,
                                        kind="ExternalOutput").ap()

    with tile.TileContext(nc) as tc:
        _emit(nc, tc, io)
    nc.compile()
    return nc


def _emit(nc, tc, io):
    # ------- long-lived pools
    cpool_cm = tc.tile_pool(name="const", bufs=1)
    spool_cm = tc.tile_pool(name="state", bufs=2)
    wpool_cm = tc.tile_pool(name="work", bufs=2)
    xpool_cm = tc.tile_pool(name="xstage", bufs=3)
    dpool_cm = tc.tile_pool(name="dram", bufs=1, space="DRAM")
    pg_cm = tc.tile_pool(name="pg", bufs=1, space="PSUM")
    pt_cm = tc.tile_pool(name="pt", bufs=2, space="PSUM")
    ps_cm = tc.tile_pool(name="ps", bufs=2, space="PSUM")
    cpool = cpool_cm.__enter__()
    spool = spool_cm.__enter__()
    wpool = wpool_cm.__enter__()
    xpool = xpool_cm.__enter__()
    dpool = dpool_cm.__enter__()
    pg = pg_cm.__enter__()
    pt = pt_cm.__enter__()
    ps = ps_cm.__enter__()

    # ---------------- constants
    ident = cpool.tile([P, P], f32)
    make_identity(nc, ident[:])
    ident_b = cpool.tile([P, P], bf16)
    nc.vector.tensor_copy(ident_b[:], ident[:])
    identr = cpool.tile([P, P], f32r)
    nc.vector.tensor_copy(identr[:], ident[:])
    ones_f = cpool.tile([1, P], f32)
    nc.vector.memset(ones_f[:], 1.0)
    ones = cpool.tile([1, P], f32r)
    nc.vector.tensor_copy(ones[:], ones_f[:])

    def load_const(name, shape):
        t = cpool.tile(list(shape), f32r, tag=name)
        nc.sync.dma_start(t[:], io[name][:])
        return t

    wps = ps.tile([P, 512], f32, tag="sc")
    for i in range(16):
        nc.tensor.matmul(wps[:, 0:P], identr[:, :], identr[:, :],
                         start=True, stop=True)

    bn0_sb = load_const("bn0", (1, H))
    bn1_sb = load_const("bn1", (1, H))
    bnd_sb = load_const("bnd", (1, H))
    fcb_sb = load_const("fcb", (1, H))
    amask_sb = cpool.tile([Bc, NSEQ], f32r, tag="amask")
    nc.sync.dma_start(amask_sb[:], io["amask"][:])

    # DRAM scratch (xp preacts, bf16)
    xp0_d = dpool.tile([NSCAN, H3], f32r)
    xp1_d = dpool.tile([NSCAN, H3], f32r)
    xpx_d = dpool.tile([NSEQ, H3], f32r)

    # ---------------- helpers
    def kload(pool, name, kdim, n, tag, dt=bf16):
        ko = kdim // P
        t = pool.tile([P, ko, n], dt, tag=tag)
        nc.sync.dma_start(t[:], io[name].rearrange("(ko p) n -> p ko n", p=P))
        return t

    def batched_mm(out_dram, lhsT_tile, kdim, mtiles, rhs_name, nbase,
                   bias_sb, opool, rdt=f32r):
        """out[m*128.., :nbase] = lhsT.T @ io[rhs_name] + bias -> DRAM f32r."""
        ko = kdim // P
        rhs_r = io[rhs_name].rearrange("(ko p) n -> p ko n", p=P)
        for c0 in range(0, nbase, 512):
            cw = min(512, nbase - c0)
            rhs_c = opool.tile([P, ko, 512], rdt, tag="rhsc")
            nc.sync.dma_start(rhs_c[:, :, :cw], rhs_r[:, :, c0:c0 + cw])
            for m in range(mtiles):
                ps_t = ps.tile([P, 512], f32, tag="sc")
                for k in range(ko):
                    nc.tensor.matmul(ps_t[:, :cw],
                                     lhsT_tile[:, k, m * P:(m + 1) * P],
                                     rhs_c[:, k, :cw],
                                     start=(k == 0), stop=False)
                nc.tensor.matmul(ps_t[:, :cw], ones[:1, :P],
                                 bias_sb[:1, c0:c0 + cw],
                                 start=False, stop=True)
                ob = opool.tile([P, 512], f32r, tag="mmob")
                nc.any.tensor_copy(ob[:, :cw], ps_t[:, :cw])
                nc.sync.dma_start(out_dram[m * P:(m + 1) * P, c0:c0 + cw],
                                  ob[:, :cw])

    def transpose_to(dst_ap, src_ap, rows):
        """dst_ap (128, rows) bf16 = src_ap (rows, 128) f32, transposed+cast."""
        tp_t = pt.tile([P, P], f32, tag="tp")
        nc.tensor.transpose(tp_t[:, :rows], src_ap, ident[:rows, :rows])
        nc.any.tensor_copy(dst_ap, tp_t[:, :rows])
        return tp_t

    # =========================================================== gather phase
    gpool_cm = tc.tile_pool(name="gather", bufs=1)
    gwork_cm = tc.tile_pool(name="gwork", bufs=2)
    gpool = gpool_cm.__enter__()
    gwork = gwork_cm.__enter__()

    xeT = kload(gpool, "xeT_in", H, NSCAN, "xeT", f32r)
    xdT = kload(gpool, "xdT_in", H, NSEQ, "xdT", f32r)

    b0_sb = gpool.tile([1, H3], f32r, tag="b0")
    nc.sync.dma_start(b0_sb[:], io["b0"][:])
    bd_sb2 = gpool.tile([1, H3], f32r, tag="bd")
    nc.sync.dma_start(bd_sb2[:], io["bd"][:])
    batched_mm(xp0_d[:], xeT, H, 6, "w0t", H3, b0_sb, gwork)
    batched_mm(xpx_d[:], xdT, H, 3, "wxd", H3, bd_sb2, gwork)

    gwork_cm.__exit__(None, None, None)
    gpool_cm.__exit__(None, None, None)

    # =========================================================== GRU scan
    #
    # Gate preacts live in ONE PSUM bank per step, at partition offsets:
    #   [0:m]   pre_r   = xp_r + h@U_r  [+ ht@Wh_r]
    #   [32:+m] pre_z   = xp_z + h@U_z  [+ ht@Wh_z]
    #   [64:+m] qn      = 0.5*(bn + h@U_n)          (U_n, bn host-prescaled)
    #   [96:+m] nx2     = xp_n + ht@Wh_n            (decoder only)
    # sigmoid(x) = (tanh(x/2)+1)/2:
    #   tr = tanh(0.5*pre_r); tz = tanh(0.5*pre_z)
    #   nin = (tr+1)*qn + nx ; n = tanh(nin)
    #   omz = 0.5 - 0.5*tz ; zh2 = (tz+1)*h
    #   h_new = (n*omz) + 0.5*zh2
    def gru_scan(nsteps, m, xp_dram, u_sb, whx_sb, bn_sb, h0_sb, h0T,
                 step_out):
        h_sb, hT = h0_sb, h0T
        for t in range(nsteps):
            whT = step_out.ht_T if whx_sb is not None else None
            xp_t = xpool.tile([m, H3], f32r, tag="xpt")
            nc.sync.dma_start(xp_t[:], xp_dram[t * m:(t + 1) * m, :])
            G = pg.tile([P, 4, 512], f32, tag="G")
            # (region_index, rhs_col_base, kind). All bf16 stationaries use
            # 128 columns (FWL path); rows >= m of G carry don't-care data.
            groups = [(0, 0, "rz"), (1, 512, "rz"), (2, 1024, "q")]
            if whx_sb is not None:
                groups.append((3, 1024, "x"))
            mms = {r: [] for r, _, _ in groups}
            for r, c0, kind in groups:
                if kind == "q":
                    mms[r].append((ones[:1, :m], bn_sb[:1, :]))
                else:
                    mms[r].append((identr[:m, :m], xp_t[:, c0:c0 + 512]))
                if hT is not None and kind != "x":
                    for k in range(4):
                        mms[r].append((hT[:, k, :m], u_sb[:, k, c0:c0 + 512]))
                if whT is not None and kind != "q":
                    for k in range(4):
                        mms[r].append((whT[:, k, :m],
                                       whx_sb[:, k, c0:c0 + 512]))
            for r, c0, kind in groups:
                last = len(mms[r]) - 1
                for i, (lh, rh) in enumerate(mms[r]):
                    nc.tensor.matmul(G[0:m, r, :], lh, rh, start=(i == 0),
                                     stop=(i == last))

            tr = wpool.tile([P, H], bf16, tag="tr")
            nc.scalar.activation(tr[0:m, :], G[0:m, 0, :], AF.Tanh,
                                 scale=0.5)
            tz = wpool.tile([P, H], bf16, tag="tz")
            nc.scalar.activation(tz[0:m, :], G[0:m, 1, :], AF.Tanh,
                                 scale=0.5)
            rq = wpool.tile([P, H], bf16, tag="rq")
            nc.vector.scalar_tensor_tensor(rq[0:m, :], tr[0:m, :], 1.0,
                                           G[0:m, 2, :], OP.add, OP.mult)
            nin = wpool.tile([P, H], bf16, tag="nin")
            if whx_sb is not None:
                nc.vector.tensor_add(nin[0:m, :], rq[0:m, :], G[0:m, 3, :])
            else:
                nc.vector.tensor_add(nin[0:m, :], rq[0:m, :],
                                     xp_t[:, H2:H3])
            n_t = wpool.tile([P, H], bf16, tag="n")
            nc.scalar.activation(n_t[0:m, :], nin[0:m, :], AF.Tanh)
            omz = wpool.tile([P, H], bf16, tag="omz")
            nc.vector.tensor_scalar(omz[0:m, :], tz[0:m, :], -0.5, 0.5,
                                    OP.mult, OP.add)
            h_new = spool.tile([P, H], f32, tag="h")
            if h_sb is not None:
                zh2 = wpool.tile([P, H], bf16, tag="zh2")
                nc.vector.scalar_tensor_tensor(zh2[0:m, :], tz[0:m, :], 1.0,
                                               h_sb[0:m, :], OP.add, OP.mult)
                t1 = wpool.tile([P, H], bf16, tag="t1")
                nc.vector.tensor_mul(t1[0:m, :], n_t[0:m, :], omz[0:m, :])
                nc.vector.scalar_tensor_tensor(h_new[0:m, :], zh2[0:m, :], 0.5,
                                               t1[0:m, :], OP.mult, OP.add)
            else:
                nc.vector.tensor_mul(h_new[0:m, :], n_t[0:m, :], omz[0:m, :])
            hT_new = spool.tile([P, 4, m], bf16, tag="hT")
            for k in range(4):
                transpose_to(hT_new[:, k, :],
                             h_new[0:m, k * P:(k + 1) * P], m)
            step_out.emit(t, h_new, hT_new)
            h_sb, hT = h_new, hT_new
        return h_sb, hT

    # mid pool outlives the encoder pools (LIFO pool order)
    mpool_cm = tc.tile_pool(name="mid", bufs=1)
    mpool = mpool_cm.__enter__()
    # hencP: encoder output in POSITION order: col = p*8 + b
    hencP = mpool.tile([P, 8, NSEQ], bf16, tag="hencP")

    # encoder layer outputs, accumulated transposed in SBUF
    e0pool_cm = tc.tile_pool(name="encp", bufs=1)
    e0work_cm = tc.tile_pool(name="encw", bufs=2)
    e0pool = e0pool_cm.__enter__()
    e0work = e0work_cm.__enter__()

    # l0S: layer-0 output in layer-1 SCAN order, transposed:
    #   plane k (0..3)=fwd dims, (4..7)=bwd dims; col r = s*16 + lane*8 + b
    l0S = e0pool.tile([P, 8, NSCAN], bf16, tag="l0S")

    class EncOut:
        ht_T = None

        def __init__(self, kind):
            self.kind = kind

        def emit(self, s, h_new, hT_new):
            if self.kind == "l0":
                nc.any.tensor_copy(l0S[:, 0:4, s * 16:s * 16 + 8],
                                   hT_new[:, :, 0:8])
                nc.any.tensor_copy(
                    l0S[:, 0:4, (T - 1 - s) * 16 + 8:(T - 1 - s) * 16 + 16],
                    hT_new[:, :, 0:8])
                nc.any.tensor_copy(
                    l0S[:, 4:8, (T - 1 - s) * 16:(T - 1 - s) * 16 + 8],
                    hT_new[:, :, 8:16])
                nc.any.tensor_copy(l0S[:, 4:8, s * 16 + 8:s * 16 + 16],
                                   hT_new[:, :, 8:16])
            else:
                nc.any.tensor_copy(hencP[:, 0:4, s * 8:s * 8 + 8],
                                   hT_new[:, :, 0:8])
                nc.any.tensor_copy(
                    hencP[:, 4:8, (T - 1 - s) * 8:(T - 1 - s) * 8 + 8],
                    hT_new[:, :, 8:16])

    # ---- encoder
    u0_sb = kload(e0pool, "u0", H, H3, "u0sb")
    gru_scan(T, 16, xp0_d[:], u0_sb, None, bn0_sb, None, None, EncOut("l0"))

    b1_sb = e0pool.tile([1, H3], f32r, tag="b1")
    nc.sync.dma_start(b1_sb[:], io["b1"][:])
    batched_mm(xp1_d[:], l0S, H2, 6, "w1t", H3, b1_sb, e0work,
               rdt=bf16)

    u1_sb = kload(e0pool, "u1", H, H3, "u0sb")   # reuse u0 slot
    # prefetch decoder recurrence weights during enc1 scan (DMA idle here)
    ud_sb = kload(mpool, "ud", H, H3, "udsb")
    gru_scan(T, 16, xp1_d[:], u1_sb, None, bn1_sb, None, None, EncOut("henc"))

    e0work_cm.__exit__(None, None, None)
    e0pool_cm.__exit__(None, None, None)

    # decoder weights in their own pool (opens after encoder pools free)
    dwpool_cm = tc.tile_pool(name="dwts", bufs=1)
    dwpool = dwpool_cm.__enter__()
    whd_sb = kload(dwpool, "whd", H, H3, "whdsb")
    wch_sb = kload(dwpool, "wch", H, H, "wchsb")

    # =========================================================== attention pre
    mwork_cm = tc.tile_pool(name="midw", bufs=2)
    mwork = mwork_cm.__enter__()

    was_sb = kload(mwork, "was", H2, H, "wpre")
    gT = mpool.tile([P, 4, NSEQ], bf16, tag="gT")
    for m in range(4):
        ps_t = ps.tile([P, 512], f32, tag="sc")
        for k in range(8):
            nc.tensor.matmul(ps_t[:, :NSEQ], was_sb[:, k, m * P:(m + 1) * P],
                             hencP[:, k, :], start=(k == 0), stop=(k == 7))
        nc.any.tensor_copy(gT[:, m, :], ps_t[:, :NSEQ])

    wcc_sb = kload(mwork, "wcc", H2, H, "wpre")
    pf = mpool.tile([P, 3, H], bf16, tag="pf")
    for m in range(3):
        ps_t = ps.tile([P, 512], f32, tag="sc")
        for k in range(8):
            nc.tensor.matmul(ps_t[:, :H], hencP[:, k, m * P:(m + 1) * P],
                             wcc_sb[:, k, :], start=(k == 0), stop=(k == 7))
        nc.any.tensor_copy(pf[:, m, :], ps_t[:, :H])

    fct_sb = kload(mwork, "fct", H2, H, "wpre")
    # fwd final state lives at cols 256..384 rows 120..128 of the product;
    # bwd final at cols 0..128 rows 0..8 (128-col bf16 stationaries only)
    h0f = ps.tile([P, 512], f32, tag="sc")
    for k in range(4):
        nc.tensor.matmul(h0f[:, :H], hencP[:, k, 2 * P:3 * P],
                         fct_sb[:, k, :], start=(k == 0), stop=(k == 3))
    h0b = ps.tile([P, 512], f32, tag="sc")
    for k in range(4, 8):
        nc.tensor.matmul(h0b[:, :H], hencP[:, k, 0:P],
                         fct_sb[:, k, :], start=(k == 4), stop=False)
    nc.tensor.matmul(h0b[:Bc, :H], ones[:1, :Bc], fcb_sb[:1, :],
                     start=False, stop=True)
    h0c32 = wpool.tile([32, H], f32, tag="h0c")
    nc.any.tensor_copy(h0c32[:], h0f[96:P, :H])
    h0c = wpool.tile([Bc, H], f32, tag="h0s")
    nc.sync.dma_start(h0c[:], h0c32[32 - Bc:32, :])
    h0_sb = spool.tile([P, H], f32, tag="h")
    nc.vector.memset(h0_sb[:], 0.0)
    h0s = wpool.tile([Bc, H], f32, tag="h0t")
    nc.vector.tensor_add(h0s[:], h0c[:], h0b[0:Bc, :H])
    nc.scalar.activation(h0_sb[0:Bc, :], h0s[:], AF.Tanh)
    h0T = spool.tile([P, 4, Bc], bf16, tag="hT")
    for k in range(4):
        transpose_to(h0T[:, k, :], h0_sb[0:Bc, k * P:(k + 1) * P], Bc)

    if io["dbg"]:
        nc.sync.dma_start(io["dbg_hencP"][:],
                          hencP[:].reshape([P, 8 * NSEQ]))
        nc.sync.dma_start(io["dbg_h0"][:], h0_sb[:])

    mwork_cm.__exit__(None, None, None)

    # =========================================================== decoder
    htall = mpool.tile([P, 4, NSEQ], bf16, tag="htall")

    class DecOut:
        ht_T = None

        def emit(self, t, h_new, hT_new):
            if io["dbg"]:
                nc.sync.dma_start(io["dbg_hall"][t * Bc:(t + 1) * Bc, :],
                                  h_new[0:Bc, :])
            sc_ps = ps.tile([P, 512], f32, tag="sc")
            for k in range(4):
                nc.tensor.matmul(sc_ps[:Bc, :NSEQ], hT_new[:, k, :],
                                 gT[:, k, :], start=(k == 0), stop=False)
            nc.tensor.matmul(sc_ps[:Bc, :NSEQ], identr[:Bc, :Bc],
                             amask_sb[:, :], start=False, stop=True)
            alpha = wpool.tile([Bc, NSEQ], f32, tag="alpha")
            sexp = wpool.tile([Bc, 1], f32, tag="sexp")
            nc.scalar.activation(alpha[:], sc_ps[:Bc, :NSEQ], AF.Exp,
                                 accum_out=sexp[:])
            rs = wpool.tile([Bc, 1], f32, tag="rs")
            nc.vector.reciprocal(rs[:], sexp[:])
            nc.vector.tensor_scalar_mul(alpha[:], alpha[:], rs[:])
            aT = wpool.tile([P, 3, Bc], bf16, tag="aT")
            for j in range(3):
                transpose_to(aT[:, j, :], alpha[:, j * P:(j + 1) * P], Bc)
            htp = ps.tile([P, 512], f32, tag="sc")
            for j in range(3):
                nc.tensor.matmul(htp[:Bc, :H], aT[:, j, :], pf[:, j, :],
                                 start=(j == 0), stop=False)
            for k in range(4):
                nc.tensor.matmul(htp[:Bc, :H], hT_new[:, k, :],
                                 wch_sb[:, k, :], start=False, stop=(k == 3))
            ht_sb = wpool.tile([Bc, H], f32, tag="hts")
            nc.scalar.activation(ht_sb[:], htp[:Bc, :H], AF.Tanh)
            ht_T = spool.tile([P, 4, Bc], bf16, tag="htT")
            for k in range(4):
                tp_t = transpose_to(ht_T[:, k, :],
                                    ht_sb[:, k * P:(k + 1) * P], Bc)
                nc.any.tensor_copy(htall[:, k, t * Bc:(t + 1) * Bc],
                                   tp_t[:, :Bc])
            self.ht_T = ht_T

    # ---------- vocab projection machinery
    prpool_cm = tc.tile_pool(name="projres", bufs=NRES)
    prpool = prpool_cm.__enter__()
    pspool_cm = tc.tile_pool(name="projstr", bufs=3)
    pspool = pspool_cm.__enter__()
    owt_r = io["owt"].rearrange("(ko p) v -> p ko v", p=P)

    res_tiles = {}

    def load_chunk(pool, tag, c):
        c0, cw = VCHUNKS[c]
        ow = pool.tile([P, 4, 512], bf16, tag=tag)
        nc.sync.dma_start(ow[:, :, :cw], owt_r[:, :, c0:c0 + cw])
        return ow

    def emit_use(ow, c, m):
        c0, cw = VCHUNKS[c]
        ps_t = ps.tile([P, 512], f32, tag="sc")
        for k in range(4):
            nc.tensor.matmul(ps_t[:, :cw], htall[:, k, m * P:(m + 1) * P],
                             ow[:, k, :cw], start=(k == 0), stop=(k == 3))
        ob = pspool.tile([P, 512], bf16, tag="ob")
        nc.any.tensor_copy(ob[:, :cw], ps_t[:, :cw])
        nc.sync.dma_start(io["out"][m * P:(m + 1) * P, c0:c0 + cw],
                          ob[:, :cw])

    pending = {c: [0, 1, 2] for c in range(len(VCHUNKS))}

    class DecOutP(DecOut):
        def emit(self, t, h_new, hT_new):
            super().emit(t, h_new, hT_new)
            if t < NRES:
                c = t
                res_tiles[c] = load_chunk(prpool, "owr", c)
            if t >= 16:
                quota = 3
                for c in range(NRES):
                    if quota == 0:
                        break
                    if c not in res_tiles or t <= c:
                        continue
                    while pending[c] and quota:
                        m = pending[c][0]
                        if t < 16 * (m + 1):
                            break
                        pending[c].pop(0)
                        emit_use(res_tiles[c], c, m)
                        quota -= 1

    dec_out = DecOutP()
    gru_scan(T, Bc, xpx_d[:], ud_sb, whd_sb, bnd_sb, h0_sb, h0T, dec_out)
    if io["dbg"]:
        nc.sync.dma_start(io["dbg_htall"][:],
                          htall[:].reshape([P, 4 * NSEQ]))
    # tail: finish resident chunks (no DMA), then stream the rest
    for c in range(NRES):
        for m in pending[c]:
            emit_use(res_tiles[c], c, m)
        pending[c] = []
    for c in range(NRES, len(VCHUNKS)):
        ow = load_chunk(pspool, "ows", c)
        for m in pending[c]:
            emit_use(ow, c, m)
        pending[c] = []
    pspool_cm.__exit__(None, None, None)
    prpool_cm.__exit__(None, None, None)
    dwpool_cm.__exit__(None, None, None)
    mpool_cm.__exit__(None, None, None)

    for cm in (ps_cm, pt_cm, pg_cm, dpool_cm, xpool_cm, wpool_cm,
               spool_cm, cpool_cm):
        cm.__exit__(None, None, None)


# ---------------------------------------------------------------- host side
_PROGRAM = None


def _get_program():
    global _PROGRAM
    if _PROGRAM is None:
        _install_profile_hook()
        _PROGRAM = build_program()
    return _PROGRAM


def _prep_shared(inputs):
    f = np.float32
    g = {}
    g["w0t"] = np.ascontiguousarray(np.asarray(inputs["enc0_Wih"], f).T)
    g["u0"] = np.concatenate([np.asarray(inputs["enc0_Ur"], f).T,
                              np.asarray(inputs["enc0_Uz"], f).T,
                              0.5 * np.asarray(inputs["enc0_Un"], f).T],
                             axis=1)
    g["b0"] = np.asarray(inputs["enc0_bih"], f)[None, :]
    g["bn0"] = 0.5 * np.asarray(inputs["enc0_bn"], f)[None, :]
    g["w1t"] = np.ascontiguousarray(np.asarray(inputs["enc1_Wih"], f).T)
    g["u1"] = np.concatenate([np.asarray(inputs["enc1_Ur"], f).T,
                              np.asarray(inputs["enc1_Uz"], f).T,
                              0.5 * np.asarray(inputs["enc1_Un"], f).T],
                             axis=1)
    g["b1"] = np.asarray(inputs["enc1_bih"], f)[None, :]
    g["bn1"] = 0.5 * np.asarray(inputs["enc1_bn"], f)[None, :]
    g["fct"] = np.ascontiguousarray(np.asarray(inputs["fc_init_w"], f).T)
    g["fcb"] = np.asarray(inputs["fc_init_b"], f)[None, :]
    scale = np.float32(1.0) / np.sqrt(np.float32(H2))
    g["was"] = np.ascontiguousarray(np.asarray(inputs["Wa"], f) * scale)
    acw = np.asarray(inputs["attn_combine_w"], f)
    g["wch"] = np.ascontiguousarray(acw[:, :H].T)
    g["wcc"] = np.ascontiguousarray(acw[:, H:].T)
    dwih = np.asarray(inputs["dec_Wih"], f)
    g["wxd"] = np.ascontiguousarray(dwih[:, :H].T)
    g["whd"] = np.ascontiguousarray(dwih[:, H:].T)
    g["bd"] = np.asarray(inputs["dec_bih"], f)[None, :]
    g["ud"] = np.concatenate([np.asarray(inputs["dec_Ur"], f).T,
                              np.asarray(inputs["dec_Uz"], f).T,
                              0.5 * np.asarray(inputs["dec_Un"], f).T],
                             axis=1)
    g["bnd"] = 0.5 * np.asarray(inputs["dec_bn"], f)[None, :]
    import ml_dtypes
    g["owt"] = np.ascontiguousarray(
        np.asarray(inputs["out_w"], f).T.astype(ml_dtypes.bfloat16))
    for k in ("u0", "w1t", "u1", "fct", "was", "wcc", "wch",
              "whd", "ud"):
        g[k] = g[k].astype(ml_dtypes.bfloat16)
    for k in g:
        g[k] = np.ascontiguousarray(g[k])
    return g


def _prep_core(inputs, c):
    src = np.asarray(inputs["src"])
    tgt = np.asarray(inputs["tgt"])
    emb = np.asarray(inputs["emb"], np.float32)
    si = src[:, c * Bc:(c + 1) * Bc].astype(np.int64)      # (48, 8)
    ti = tgt[:, c * Bc:(c + 1) * Bc].astype(np.int64)
    idx_enc = np.empty((T, 2, Bc), np.int64)
    idx_enc[:, 0, :] = si
    idx_enc[:, 1, :] = si[::-1]
    xeT_in = np.ascontiguousarray(emb[idx_enc.reshape(NSCAN)].T)
    xdT_in = np.ascontiguousarray(emb[ti.reshape(NSEQ)].T)
    m = np.full((Bc, T, Bc), NEG, np.float32)
    for b in range(Bc):
        m[b, :, b] = np.where(si[:, b] != 0, np.float32(0.0), np.float32(NEG))
    return {"xeT_in": xeT_in,
            "xdT_in": xdT_in,
            "amask": m.reshape(Bc, NSEQ)}


def kernel(**inputs):
    nc = _get_program()
    shared = _prep_shared(inputs)
    in_maps = []
    for c in range(NCORES):
        im = dict(shared)
        im.update(_prep_core(inputs, c))
        in_maps.append(im)
    res = run_bass_kernel_spmd(nc, in_maps, core_ids=list(range(NCORES)))
    logits = np.empty((T, B, V), np.float32)
    for c in range(NCORES):
        logits[:, c * Bc:(c + 1) * Bc, :] = \
            np.asarray(res.results[c]["out"], np.float32).reshape(T, Bc, V)
    return logits
